# revision 3
# baseline (speedup 1.0000x reference)
"""Trainium2 Bass kernel for nn_AssociationScore (GCN + MLP scoring head).

The computation is linear up to the final sigmoid, so the 64-dim GCN
aggregation collapses to a per-node scalar:
    w3  = W @ w2                       (256-vector, computed on device)
    u   = x @ w3                       (per-node scalar matvec, bf16 x)
    g   = dinv * u                     (dinv = rsqrt(indeg + 1))
    Z[d] = sum over edges (s->d) of g[s]
    score = sigmoid(c0 + dinv*(Z + dinv*u)),  c0 = b@w2 + b2

Sharding: nodes row-sharded over 8 NeuronCores (12500/core, padded to
12544 = 98*128; local node n = t*128 + p lives at [partition p, slot t]).
Each core computes its u/g shard, all-gathers g (p-major layout so the
DRAM write is contiguous), then aggregates the edges whose dst lies in
its shard:

  * per-(core, src-octant) dst-sorted edge streams gather g[src] via
    GPSIMD ap_gather (20 chunks; the per-index SBUF read-command cost of
    ~27ns is the hardware floor and sets the kernel's critical path);
  * the segment-sum runs on the TensorEngine, pipelined under the
    gather: each 128-slot subchunk is PE-transposed (slots onto
    partitions) and contracted against host-shipped fp8 selection
    blocks (entry = edge count for (slot, dst)), accumulating per
    dst-tile in PSUM.  The matmul schedule is the union over cores of
    (octant, subchunk, tile) triples, emitted tile-major so only one
    PSUM accumulation group is open at a time; cores without edges for
    an entry ship a zero block.

Host-side work is integer routing only (sort + counts + int16 index
tables + fp8 selection blocks); all floating-point math runs on device.
"""
import numpy as np
import ml_dtypes

NCORES = 8
N = 100000
M = 12500
MP = 12544           # 98 * 128
JT = 98              # dst tiles per core (128 dsts each)
D = 256
H = 64
NCH = 20             # gather chunks
TB = 7


def _routing(src, dst):
    """Integer routing + bf16 selection blocks.

    Streams: per (core, src-octant) dst-sorted edge streams (as v1), slot
    indices wrap16-ed int16 for ap_gather.  Selection schedule: for each
    128-slot subchunk of each stream, the edges map to dst tiles; emit one
    [128, 128] bf16 count block per (octant, subchunk, tile) triple.
    """
    core = dst // M
    octv = src // M
    nloc = dst - core * M
    # local node id n = t*128 + p  (t = tile, p = partition within tile)
    key = (core * 8 + octv) * np.int64(N) + nloc
    order = np.argsort(key, kind='stable')
    s_s = src[order]
    grp = (core * 8 + octv)[order]
    nloc_s = nloc[order]
    counts = np.bincount(grp, minlength=64)
    KE = int(counts.max())
    KE = ((KE + NCH * 128 - 1) // (NCH * 128)) * (NCH * 128)
    assert KE <= 32768
    offs = np.concatenate([[0], np.cumsum(counts)])
    slotpos = np.arange(len(order)) - offs[grp]          # slot in stream

    PADIDX = (12543 % 128) * JT + 12543 // 128           # pad node, g = 0
    idx_main = np.full((NCORES, 8, KE), PADIDX, np.int16)
    sl = s_s % M
    idx_main.reshape(64, KE)[grp, slotpos] = ((sl % 128) * JT + sl // 128).astype(np.int16)

    # selection blocks, UNION schedule over cores so the SPMD program is
    # core-invariant: one (oct, subchunk, tile) entry if ANY core has edges
    # there; cores without edges ship a zero block at that position.
    sub = slotpos // 128
    kk = slotpos % 128
    tt = nloc_s // 128
    pp = nloc_s % 128
    NSUB = KE // 128
    oct_s = grp % 8
    core_s = grp // 8
    ukey = (oct_s * NSUB + sub) * np.int64(JT) + tt      # core-independent
    ub0 = np.unique(ukey)                                # sorted
    ub_oct = (ub0 // (np.int64(NSUB) * JT)).astype(np.int64)
    ub_sub = ((ub0 // JT) % NSUB).astype(np.int64)
    ub_tile = (ub0 % JT).astype(np.int64)
    # tile-major order: one psum accumulation group open at a time
    o2 = np.lexsort((ub_sub, ub_oct, ub_tile))
    ub_oct, ub_sub, ub_tile = ub_oct[o2], ub_sub[o2], ub_tile[o2]
    nblk = len(ub0)
    inv_o2 = np.empty(nblk, np.int64)
    inv_o2[o2] = np.arange(nblk)
    sched = list(zip(ub_oct.tolist(), ub_sub.tolist(), ub_tile.tolist(),
                     range(nblk)))

    bi = inv_o2[np.searchsorted(ub0, ukey)]
    SELW = 32
    NSB = (nblk + SELW - 1) // SELW
    nblk_pad = NSB * SELW
    selpercore = []
    for c in range(NCORES):
        m = core_s == c
        flat = bi[m] * (128 * 128) + kk[m] * 128 + pp[m]
        selcnt = np.bincount(flat, minlength=nblk_pad * 128 * 128)
        blks = selcnt.reshape(NSB, SELW, 128, 128)
        # superblock-partition-major: [NSB, 128 slot-partitions, SELW*128]
        selpercore.append(np.ascontiguousarray(
            blks.transpose(0, 2, 1, 3).reshape(NSB, 128, SELW * 128))
            .astype(ml_dtypes.float8_e4m3fn))

    def wrap16(a):                                  # [8, K] -> [128, K//16]
        o, K = a.shape
        return np.ascontiguousarray(
            a.reshape(o, K // 16, 16).transpose(0, 2, 1).reshape(o * 16, K // 16))

    idxm = np.stack([wrap16(idx_main[c]) for c in range(NCORES)])
    deg = (np.bincount(dst, minlength=N) + 1).astype(np.float32)
    return idxm, sched, selpercore, deg, KE


def _emit(nc, tc, t, KE, sched, nblk):
    import concourse.mybir as mybir
    import concourse.tile as tile  # noqa: F401

    dt = mybir.dt
    f32 = dt.float32
    bf16 = dt.bfloat16
    fp8 = dt.float8e4
    Alu = mybir.AluOpType
    NSUB = KE // 128
    CH = KE // NCH
    SUBPC = CH // 128            # subchunks per gather chunk

    # first/last block per tile for psum start/stop
    first_of_tile, last_of_tile = {}, {}
    for (o, sb, tl, bi) in sched:
        if tl not in first_of_tile:
            first_of_tile[tl] = bi
        last_of_tile[tl] = bi

    # group schedule by subchunk
    by_sub = {}
    for (o, sb, tl, bi) in sched:
        by_sub.setdefault(sb, []).append((o, tl, bi))

    SELW = 32                    # sel blocks per DMA superblock
    NSB = (nblk + SELW - 1) // SELW

    with tc.tile_pool(name="const", bufs=1) as cpool, \
         tc.tile_pool(name="xload", bufs=2) as xpool, \
         tc.tile_pool(name="scr", bufs=1) as spool, \
         tc.tile_pool(name="gchp", bufs=2) as gpool, \
         tc.tile_pool(name="selp", bufs=3) as selpool, \
         tc.tile_pool(name="xcp", bufs=16) as xcpool, \
         tc.tile_pool(name="ps", bufs=1, space="PSUM") as ppool, \
         tc.tile_pool(name="pst", bufs=2, space="PSUM") as tpool, \
         tc.tile_pool(name="dram", bufs=1, space="DRAM") as dpool:

        # ---- constants
        identt = cpool.tile([128, 128], f32)
        nc.sync.dma_start(identt[:], t["ident"])
        wTt = cpool.tile([H, D], f32)
        nc.sync.dma_start(wTt[:], t["wT"])
        w2rt = cpool.tile([H, 128], f32)
        nc.sync.dma_start(w2rt[:], t["w2r"])
        bcolt = cpool.tile([H, 1], f32)
        nc.sync.dma_start(bcolt[:], t["bcol"])
        b2rt = cpool.tile([128, 1], f32)
        nc.sync.dma_start(b2rt[:], t["b2r"])
        idxmt = cpool.tile([128, KE // 16], dt.int16)
        nc.sync.dma_start(idxmt[:], t["idxm"])
        degt = cpool.tile([128, JT], f32)
        nc.sync.dma_start(degt[:], t["deg98"])

        # ---- w3 replicated, c0
        w3ps = ppool.tile([128, D], f32, tag="w3ps")
        nc.tensor.matmul(w3ps[:], w2rt[:], wTt[:], start=True, stop=True)
        w3rep = cpool.tile([128, D], f32)
        nc.vector.tensor_copy(w3rep[:], w3ps[:])
        c0ps = ppool.tile([128, 1], f32, tag="c0ps")
        nc.tensor.matmul(c0ps[:], w2rt[:], bcolt[:], start=True, stop=True)
        c0t = cpool.tile([128, 1], f32)
        nc.vector.tensor_add(c0t[:], c0ps[:], b2rt[:])

        # ---- matvec u = xs @ w3 -> U [128, JT]   (node n = j*128+p at U[p, j])
        w3b = cpool.tile([128, D], bf16)
        nc.vector.tensor_copy(w3b[:], w3rep[:])
        U = spool.tile([128, JT], f32)
        xsv = t["xs"].rearrange("(b a p) k -> b p a k", p=128, a=TB)
        for bb in range(JT // TB):
            xt = xpool.tile([128, TB * D], bf16, tag="xt")
            nc.sync.dma_start(xt[:].rearrange("p (a k) -> p a k", k=D), xsv[bb])
            xt3 = xt[:].rearrange("p (a k) -> p a k", k=D)
            nc.vector.tensor_mul(
                xt3, xt3,
                w3b[:].rearrange("p (o k) -> p o k", o=1)
                .broadcast_to([128, TB, D]))
            nc.vector.tensor_reduce(
                U[:, bb * TB:(bb + 1) * TB], xt3,
                axis=mybir.AxisListType.X, op=Alu.add)

        # ---- dinv, g
        rec = spool.tile([128, JT], f32)
        nc.vector.reciprocal(rec[:], degt[:])
        dinvt = spool.tile([128, JT], f32)
        nc.scalar.sqrt(dinvt[:], rec[:])
        gblk = spool.tile([128, JT], f32)
        nc.vector.tensor_mul(gblk[:], dinvt[:], U[:])

        # ---- allgather g
        gshard = dpool.tile([1, MP], f32)
        nc.sync.dma_start(
            gshard.opt()[0].rearrange("(p j) -> p j", p=128), gblk[:])
        gfull = dpool.tile([NCORES, MP], f32)
        nc.gpsimd.collective_compute(
            "AllGather", Alu.bypass,
            replica_groups=[list(range(NCORES))],
            ins=[gshard.opt()], outs=[gfull.opt()])

        # ---- octant tables: only partition 16o is ever read post-transpose,
        # so load each octant's shard into that single partition; the other 15
        # partitions stay zero (memzero keeps the transposed garbage finite).
        gtab = spool.tile([128, MP], f32, tag="gtb")
        nc.gpsimd.memset(gtab[:], 0.0)
        for o in range(8):
            nc.sync.dma_start(gtab[16 * o:16 * o + 1, :], gfull.opt()[o:o + 1, :])

        # ---- psum accumulator for Z [128, JT]
        Zps = ppool.tile([128, JT], f32, tag="zps")

        # ---- tile-major: emit gather chunks / transposes on demand
        gtab3 = gtab[:].rearrange("p (n d) -> p n d", d=1)
        seltiles = {}
        gchunks = {}
        xcs = {}
        next_chunk = 0

        def ensure_chunk(ci):
            nonlocal next_chunk
            while next_chunk <= ci:
                gch = gpool.tile([128, CH], f32, tag="gch")
                nc.gpsimd.ap_gather(
                    out_ap=gch[:].rearrange("p (n d) -> p n d", d=1),
                    in_ap=gtab3,
                    idxs_ap=idxmt[:, next_chunk * (CH // 16):
                                  (next_chunk + 1) * (CH // 16)],
                    channels=128, num_elems=MP, d=1,
                    num_idxs=CH)
                gchunks[next_chunk] = gch
                next_chunk += 1

        def ensure_xc(sb):
            if sb in xcs:
                return xcs[sb]
            ci = sb // SUBPC
            ensure_chunk(ci)
            gch = gchunks[ci]
            ss = sb % SUBPC
            ttp = tpool.tile([128, 128], f32, tag="tt")
            nc.tensor.transpose(ttp[:], gch[:, ss * 128:(ss + 1) * 128],
                                identt[:])
            xc = xcpool.tile([128, 8], bf16, tag="xc")
            nc.vector.tensor_copy(xc[:], ttp[:, 0:128:16])
            xcs[sb] = xc
            # drop stale entries so old pool buffers can rotate
            for k in [k for k in xcs if k < sb - 24]:
                del xcs[k]
            return xc

        for (o, sb, tl, bi) in sched:
            xc = ensure_xc(sb)
            sbk = bi // SELW
            if sbk not in seltiles:
                st = selpool.tile([128, SELW * 128], fp8, tag="sel")
                nc.sync.dma_start(st[:], t["sel"][sbk])
                seltiles = {sbk: st}       # keep only newest
            st = seltiles[sbk]
            loc = bi % SELW
            nc.tensor.matmul(
                Zps[:, tl:tl + 1],
                st[:, loc * 128:(loc + 1) * 128],
                xc[:, o:o + 1],
                start=(first_of_tile[tl] == bi),
                stop=(last_of_tile[tl] == bi))
        ensure_chunk(NCH - 1)

        # ---- combine, sigmoid
        Zsb = spool.tile([128, JT], f32)
        nc.vector.tensor_copy(Zsb[:], Zps[:])
        t1 = spool.tile([128, JT], f32)
        nc.vector.tensor_add(t1[:], Zsb[:], gblk[:])
        t2 = spool.tile([128, JT], f32)
        nc.vector.tensor_mul(t2[:], dinvt[:], t1[:])
        res = spool.tile([128, JT], f32)
        nc.scalar.activation(res[:], t2[:],
                             mybir.ActivationFunctionType.Sigmoid,
                             bias=c0t[:])
        nc.sync.dma_start(t["out"].rearrange("(p j) -> p j", p=128), res[:])


def _build_nc(KE, sched, nblk):
    import concourse.bacc as bacc
    import concourse.mybir as mybir
    import concourse.tile as tile

    dt = mybir.dt
    f32 = dt.float32
    nc = bacc.Bacc("TRN2", target_bir_lowering=False, debug=False,
                   num_devices=NCORES)
    SELW = 32
    nblk_pad = ((nblk + SELW - 1) // SELW) * SELW
    t = {
        "xs": nc.dram_tensor("xs", [MP, D], dt.bfloat16, kind="ExternalInput").ap(),
        "wT": nc.dram_tensor("wT", [H, D], f32, kind="ExternalInput").ap(),
        "w2r": nc.dram_tensor("w2r", [H, 128], f32, kind="ExternalInput").ap(),
        "bcol": nc.dram_tensor("bcol", [H, 1], f32, kind="ExternalInput").ap(),
        "b2r": nc.dram_tensor("b2r", [128, 1], f32, kind="ExternalInput").ap(),
        "deg98": nc.dram_tensor("deg98", [128, JT], f32, kind="ExternalInput").ap(),
        "ident": nc.dram_tensor("ident", [128, 128], f32, kind="ExternalInput").ap(),
        "idxm": nc.dram_tensor("idxm", [128, KE // 16], dt.int16, kind="ExternalInput").ap(),
        "sel": nc.dram_tensor("sel", [nblk_pad // 32, 128, 32 * 128], dt.float8e4, kind="ExternalInput").ap(),
        "out": nc.dram_tensor("out", [MP], f32, kind="ExternalOutput").ap(),
    }
    with tile.TileContext(nc) as tc:
        _emit(nc, tc, t, KE, sched, nblk_pad)
    nc.compile()
    return nc


def _make_in_maps(x, edge_index, W, b, w2, b2):
    src = np.asarray(edge_index[0], dtype=np.int64)
    dst = np.asarray(edge_index[1], dtype=np.int64)
    idxm, sched, selpercore, deg, KE = _routing(src, dst)

    xf = np.asarray(x, dtype=np.float32)
    Wf = np.asarray(W, dtype=np.float32)
    w2f = np.asarray(w2, dtype=np.float32).reshape(H)
    bf = np.asarray(b, dtype=np.float32)
    b2f = np.asarray(b2, dtype=np.float32).reshape(1)

    wT = np.ascontiguousarray(Wf.T)
    w2rep = np.ascontiguousarray(np.broadcast_to(w2f.reshape(H, 1), (H, 128)))
    bcol = bf.reshape(H, 1)
    b2rep = np.full((128, 1), float(b2f[0]), np.float32)
    identm = np.eye(128, dtype=np.float32)

    # xs row r holds local node n = j*128+p, p=r%128, j=7*(r//896)+(r%896)//128
    r = np.arange(MP)
    jr = 7 * (r // 896) + (r % 896) // 128
    nr = jr * 128 + (r % 128)
    nn = np.arange(MP)

    SELW = 32
    nblk_pad = ((len(sched) + SELW - 1) // SELW) * SELW

    in_maps = []
    for c in range(NCORES):
        xsp = np.zeros((MP, D), ml_dtypes.bfloat16)
        valid = nr < M
        xsp[valid] = xf[c * M + nr[valid]].astype(ml_dtypes.bfloat16)
        degp = np.ones((128, JT), np.float32)
        degp[nn % 128, nn // 128] = np.where(
            nn < M, deg[c * M + np.minimum(nn, M - 1)], 1.0)
        selc = selpercore[c]
        in_maps.append({
            "xs": xsp,
            "wT": wT,
            "w2r": w2rep,
            "bcol": bcol,
            "b2r": b2rep,
            "deg98": degp,
            "ident": identm,
            "idxm": idxm[c],
            "sel": selc,
        })
    return in_maps, KE, sched, nblk_pad


def _unshard(outv):
    nn = np.arange(M)
    pos = (nn % 128) * JT + nn // 128
    return np.concatenate([outv[c][pos] for c in range(NCORES)]).astype(np.float32)


def kernel(x, edge_index, W, b, w2, b2):
    in_maps, KE, sched, nblk_pad = _make_in_maps(x, edge_index, W, b, w2, b2)
    nc = _build_nc(KE, sched, nblk_pad)
    from concourse.bass_utils import run_bass_kernel_spmd
    res = None
    for attempt in range(3):
        try:
            res = run_bass_kernel_spmd(nc, in_maps, list(range(NCORES)))
            break
        except Exception:
            if attempt == 2:
                raise
            import jax
            import jax.numpy as jnp
            a = np.eye(128, dtype=np.float32)
            for d in jax.devices()[:NCORES]:
                jnp.dot(jax.device_put(a, d), jax.device_put(a, d)).block_until_ready()
    outv = np.stack([res.results[c]["out"] for c in range(NCORES)])
    return _unshard(outv)
